# revision 30
# baseline (speedup 1.0000x reference)
"""DeepSeekMoE (B=8,S=4096,D=1024,H=512,E=8,top-2) Trainium2 kernel.

Strategy (8 NeuronCores, SPMD data-parallel over tokens, 4096 tokens/core):

 * Host: router only (logits + top-2 + softmax, executed with the exact same
   jax ops as the reference on CPU).  The smallest top2-vs-top3 logit margin
   in this problem is 4e-7 -- ANY reordered fp32 matmul flips that token's
   routing with ~50% probability, so routing decisions must be bit-identical
   to the reference.  The router is 0.25% of total FLOPs.  The host also
   pre-gathers tokens into per-expert slots (the "all-to-all dispatch" done
   at sharding time) and pre-transposes to d-major.
 * Tokens are assigned to 16 half-shard bins (2 per core) by a greedy
   per-pair-class deal that equalizes every expert's load across bins, so
   the runtime-BAKED per-(core,expert) slot capacity (rounded up to 128)
   is within ~1 tile of the theoretical minimum: ~8.7k routed slots/core
   for seed-0 data vs 10.2k with the old fixed CAP=1280 scheme.
 * Device (per core): shared-expert MLP over all 4096 tokens writes y
   directly (fp16); 8 routed expert MLPs over slot columns; exact-erf GELU
   on the scalar engine; per-slot gate scaling on the vector engine; combine
   via fp16 dma_scatter_add (SWDGE CCE) into y.  fp16 RMW halves the
   scatter traffic vs fp32.
 * fp16 on the PE (full 78.6 TF/s rate).  Weights pre-scaled by 1024 on the
   host so W~N(0,0.02) lands in fp16 normal range; the 2^-10 rescale is
   folded into the GELU activation scale (layer 1) and into the gate values
   (layer 2; 1/1024 folded into the shared-path output scale as well).
"""

import os
import numpy as np

# ---- problem constants (hardcoded; kernel.py must be self-contained) ----
B, S, D, H, E = 8, 4096, 1024, 512, 8
TOP_K = 2
N_CORES = 8
N = B * S                  # 32768 tokens total
T = N // N_CORES           # 4096 tokens per core
HT = T // 2                # tokens per half-shard bin
WSCALE = 1024.0
INV_WSCALE = float(1.0 / WSCALE)

_CACHE = {}


def _build_nc(te):
    """Build + schedule the per-core Bass program (same NEFF for all cores).

    te: tuple of E ints -- 128-slot tiles per routed expert (runtime-baked).
    """
    import concourse.bacc as bacc
    import concourse.mybir as mybir
    import concourse.tile as tile

    dt = mybir.dt
    se = [t * 128 for t in te]          # padded slot columns per expert
    eoff = np.concatenate([[0], np.cumsum(se)]).astype(int)
    S_all = int(eoff[-1])
    NT = int(sum(te))                   # total 128-slot tiles

    nc = bacc.Bacc("TRN2", target_bir_lowering=False, debug=False,
                   num_devices=N_CORES)

    # DRAM I/O.  Layouts chosen so every SBUF tile is a plain slice:
    #   xs  [c, p, b, s] = x[token c*512+s, d=b*128+p]   (shared path)
    #   xg  [p, b, s]    = x[token of slot s, d=b*128+p] (slot s, 0 for pads)
    #   w1  [e, p, b, h] = 1024*W1r[e, b*128+p, h]          (lhsT layout)
    #   w2  [e, p, m, d] = 1024*W2r[e, m*128+p, d]          (rhs layout)
    #   gates [p, c]     = g(slot s=c*128+p) / 1024         (0 for pad slots)
    #   sidx  [p, v]     = token row of slot s=v*16+p, or T (int16, wrapped-16)
    xs = nc.dram_tensor("xs", [T // 512, 128, 8, 512], dt.float16,
                        kind="ExternalInput")
    xg = nc.dram_tensor("xg", [128, 8, S_all], dt.float16,
                        kind="ExternalInput")
    w1 = nc.dram_tensor("w1", [E, 128, 8, H], dt.float16, kind="ExternalInput")
    w2 = nc.dram_tensor("w2", [E, 128, 4, 1024], dt.float16,
                        kind="ExternalInput")
    w1s = nc.dram_tensor("w1s", [128, 8, H], dt.float16, kind="ExternalInput")
    w2s = nc.dram_tensor("w2s", [128, 4, 1024], dt.float16,
                         kind="ExternalInput")
    gates = nc.dram_tensor("gates", [128, NT], dt.float32,
                           kind="ExternalInput")
    sidx = nc.dram_tensor("sidx", [128, S_all // 16], dt.int16,
                          kind="ExternalInput")
    # fp16 output; one extra trash row absorbs pad-slot scatters
    y = nc.dram_tensor("y", [T + 1, 1024], dt.float16, kind="ExternalOutput")

    GELU = mybir.ActivationFunctionType.Gelu
    MULT = mybir.AluOpType.mult

    with tile.TileContext(nc) as tc:
        with (
            tc.tile_pool(name="xpool", bufs=3) as xpool,
            tc.tile_pool(name="xgpool", bufs=2) as xgpool,
            tc.tile_pool(name="w1pool", bufs=2) as w1pool,
            tc.tile_pool(name="w2pool", bufs=2) as w2pool,
            tc.tile_pool(name="hpool", bufs=3) as hpool,
            tc.tile_pool(name="ypool", bufs=3) as ypool,
            tc.tile_pool(name="spool", bufs=4) as spool,
            tc.tile_pool(name="meta", bufs=1) as meta,
            tc.tile_pool(name="ph", bufs=2, space="PSUM") as ph_pool,
            tc.tile_pool(name="po", bufs=2, space="PSUM") as po_pool,
        ):
            w1s_sb = meta.tile([128, 8, H], dt.float16)
            nc.sync.dma_start(w1s_sb[:], w1s[:])
            w2s_sb = meta.tile([128, 4, 1024], dt.float16)
            nc.sync.dma_start(w2s_sb[:], w2s[:])
            gates_sb = meta.tile([128, NT], dt.float32)
            sidx_sb = meta.tile([128, S_all // 16], dt.int16)

            def mlp_chunk(x_tile, cs, w1_sb, w2_sb, emit_out):
                """One <=512-token chunk (cs multiple of 128): L1 + GELU + L2;
                emit_out(tt, psum_o) consumes each 128-token layer-2 tile."""
                h16 = hpool.tile([128, 4, 512], dt.float16, tag="h16")
                for mp in range(2):
                    psum_h = ph_pool.tile([128, 2, 512], dt.float32, tag="ph")
                    for mi in range(2):
                        m = mp * 2 + mi
                        for b in range(8):
                            nc.tensor.matmul(
                                psum_h[:, mi, :cs],
                                lhsT=w1_sb[:, b, m * 128:(m + 1) * 128],
                                rhs=x_tile[:, b, :cs],
                                start=(b == 0), stop=(b == 7),
                            )
                        nc.scalar.activation(h16[:, m, :cs], psum_h[:, mi, :cs],
                                             GELU, scale=INV_WSCALE)
                for tt in range(cs // 128):
                    psum_o = po_pool.tile([128, 1024], dt.float32, tag="po")
                    for half in range(2):
                        for m in range(4):
                            nc.tensor.matmul(
                                psum_o[:, half * 512:(half + 1) * 512],
                                lhsT=h16[:, m, tt * 128:(tt + 1) * 128],
                                rhs=w2_sb[:, m, half * 512:(half + 1) * 512],
                                start=(m == 0), stop=(m == 3),
                            )
                    emit_out(tt, psum_o)

            # ---- shared expert: dense over all T tokens, writes y (fp16).
            # These plain slice-writes are pairwise disjoint (parallel); the
            # routed scatter-adds below RMW the whole tensor, so Tile orders
            # them after every shared write -- exactly the required ordering.
            for c in range(T // 512):
                x_tile = xpool.tile([128, 8, 512], dt.float16, tag="xt")
                nc.sync.dma_start(x_tile[:], xs[c])

                def emit_shared(tt, psum_o, c=c):
                    y_sb = ypool.tile([128, 1024], dt.float16, tag="ysh")
                    nc.vector.tensor_scalar(y_sb[:], psum_o[:], INV_WSCALE,
                                            None, op0=MULT)
                    row = (c * 4 + tt) * 128
                    nc.sync.dma_start(y[row:row + 128, :], y_sb[:])

                mlp_chunk(x_tile, 512, w1s_sb, w2s_sb, emit_shared)

            # ---- routed experts: runtime-sized slots, fp16 scatter-add ----
            nc.sync.dma_start(gates_sb[:], gates[:])
            nc.sync.dma_start(sidx_sb[:], sidx[:])
            for e in range(E):
                w1_sb = w1pool.tile([128, 8, H], dt.float16, tag="w1")
                nc.sync.dma_start(w1_sb[:], w1[e])
                w2_sb = w2pool.tile([128, 4, 1024], dt.float16, tag="w2")
                nc.sync.dma_start(w2_sb[:], w2[e])

                xg_tile = xgpool.tile([128, 8, se[e]], dt.float16, tag="xg")
                nc.sync.dma_start(xg_tile[:],
                                  xg[:, :, int(eoff[e]):int(eoff[e + 1])])

                # slot-tiles batch into <=4-tile scatter groups
                bdefs = []  # (tile0, ntiles)
                t0, rem = 0, te[e]
                while rem > 0:
                    nb = min(4, rem)
                    bdefs.append((t0, nb))
                    t0 += nb
                    rem -= nb
                tile2b = {}
                for bi, (t0b, nb) in enumerate(bdefs):
                    for j in range(nb):
                        tile2b[t0b + j] = (bi, j)
                ysc_b = [spool.tile([128, 4, 1024], dt.float16, tag="ysc",
                                    name=f"ysc_e{e}b{bi}")
                         for bi in range(len(bdefs))]
                gbase = int(sum(te[:e]))
                done = 0
                while done < se[e]:
                    cs = min(512, se[e] - done)

                    def emit_routed(tt, psum_o, col=done, e=e):
                        gtile = col // 128 + tt
                        bi, lt = tile2b[gtile]
                        nc.vector.tensor_scalar(
                            ysc_b[bi][:, lt, :], psum_o[:],
                            gates_sb[:, gbase + gtile:gbase + gtile + 1],
                            None, op0=MULT)

                    mlp_chunk(xg_tile[:, :, done:done + cs], cs,
                              w1_sb, w2_sb, emit_routed)
                    done += cs

                for bi, (t0b, nb) in enumerate(bdefs):
                    ns = nb * 128
                    vbase = (int(eoff[e]) + t0b * 128) // 16
                    nc.gpsimd.dma_scatter_add(
                        y[:, :], ysc_b[bi][:, :nb, :],
                        sidx_sb[:, vbase:vbase + ns // 16],
                        ns, ns, 1024,
                    )

    nc.compile()
    return nc


def _routing(xf, Wg, gate_bias):
    """Bit-exact replication of the reference router on jax CPU."""
    import jax
    import jax.numpy as jnp

    cpu = jax.devices("cpu")[0]
    with jax.default_device(cpu):
        xj = jnp.asarray(np.asarray(xf), dtype=jnp.float32)
        logits = xj @ jnp.asarray(np.asarray(Wg)) + jnp.asarray(
            np.asarray(gate_bias))
        top_v, top_i = jax.lax.top_k(logits, TOP_K)
        gw = jax.nn.softmax(top_v, axis=-1)
    return np.asarray(top_i), np.asarray(gw, np.float32)


def _balance(top_i):
    """Assign tokens to 16 half-shard bins (2048 each) equalizing every
    expert's per-bin load.  Returns [16][2048] sorted original token ids."""
    NB = 2 * N_CORES
    pairs = np.sort(top_i, axis=1)
    key = pairs[:, 0] * E + pairs[:, 1]
    bins = [[] for _ in range(NB)]
    tot = np.zeros(NB, int)
    eload = np.zeros((NB, E), int)
    for k in np.unique(key):
        toks = np.nonzero(key == k)[0]
        a, b = int(k) // E, int(k) % E
        m = len(toks)
        base, rem = divmod(m, NB)
        order = np.lexsort((eload[:, a] + eload[:, b], tot))
        cnt = np.full(NB, base)
        cnt[order[:rem]] += 1
        p = 0
        for bi in range(NB):
            c = int(cnt[bi])
            bins[bi].extend(toks[p:p + c].tolist())
            p += c
            tot[bi] += c
            eload[bi, a] += c
            eload[bi, b] += c
    for _ in range(N):
        hi, lo = int(tot.argmax()), int(tot.argmin())
        if tot[hi] == HT:
            break
        best = None
        for idx, t in enumerate(bins[hi]):
            a, b = pairs[t]
            score = (eload[hi, a] - eload[lo, a]) + (eload[hi, b] - eload[lo, b])
            if best is None or score > best[0]:
                best = (score, idx, int(a), int(b))
        _, idx, a, b = best
        t = bins[hi].pop(idx)
        bins[lo].append(t)
        tot[hi] -= 1
        tot[lo] += 1
        eload[hi, a] -= 1
        eload[hi, b] -= 1
        eload[lo, a] += 1
        eload[lo, b] += 1
    assert (tot == HT).all(), tot
    return [np.sort(np.asarray(b, np.int64)) for b in bins], eload


def _prep_core(xc, top_i, gw, te):
    """Build per-core device inputs for one 4096-token shard."""
    t = xc.shape[0]
    se = [k * 128 for k in te]
    eoff = np.concatenate([[0], np.cumsum(se)]).astype(int)
    S_all = int(eoff[-1])
    NT = int(sum(te))
    xs = np.ascontiguousarray(
        xc.T.astype(np.float16).reshape(8, 128, t // 512, 512)
        .transpose(2, 1, 0, 3))
    xg = np.zeros((128, 8, S_all), np.float16)
    gates = np.zeros((128, NT), np.float32)
    sidx = np.full((16, S_all // 16), t, np.int16)  # pads -> trash row
    for e in range(E):
        ksel = top_i == e                      # [t, 2]
        rows = np.nonzero(ksel.any(1))[0]
        g = (gw * ksel).sum(1)[rows].astype(np.float32)
        n = len(rows)
        assert n <= se[e], f"expert {e}: {n} > {se[e]}"
        gt = xc[rows].T.astype(np.float16)     # [1024, n]
        xg[:, :, int(eoff[e]):int(eoff[e]) + n] = (
            gt.reshape(8, 128, n).transpose(1, 0, 2))
        s = int(eoff[e]) + np.arange(n)
        gates[s % 128, s // 128] = g * INV_WSCALE
        sidx[s % 16, s // 16] = rows.astype(np.int16)
    return {"xs": xs, "xg": xg, "gates": gates,
            "sidx": np.ascontiguousarray(np.tile(sidx, (8, 1)))}


def _ensure_ntff_hook():
    """This image's antenv lacks axon_hooks; register the NTFF-profile hook
    (used only when KERNEL_TRACE=1) via the documented ctypes path."""
    import sys
    import types
    try:
        import antenv.axon_hooks  # noqa: F401
        return
    except ImportError:
        pass
    mod = types.ModuleType("antenv.axon_hooks")
    _h = [None]
    mod.set_axon_ntff_profile_hook = lambda h: _h.__setitem__(0, h)
    mod.get_axon_ntff_profile_hook = lambda: _h[0]
    sys.modules["antenv.axon_hooks"] = mod
    try:
        import antenv
        antenv.axon_hooks = mod
        from trn_agent_boot.trn_boot import _ntff_profile_via_ctypes
        mod.set_axon_ntff_profile_hook(
            _ntff_profile_via_ctypes("/opt/axon/libaxon_pjrt.so"))
    except Exception:
        pass  # hook stays None -> concourse skips tracing gracefully


def kernel(**inputs):
    from concourse.bass_utils import run_bass_kernel_spmd
    _ensure_ntff_hook()

    x = np.asarray(inputs["x"], np.float32)
    Wg = np.asarray(inputs["Wg"], np.float32)
    gate_bias = np.asarray(inputs["gate_bias"], np.float32)
    W1s = np.asarray(inputs["W1s"], np.float32)
    W2s = np.asarray(inputs["W2s"], np.float32)
    b2s = np.asarray(inputs["b2s"], np.float32)
    W1r = np.asarray(inputs["W1r"], np.float32)
    W2r = np.asarray(inputs["W2r"], np.float32)
    b2r = np.asarray(inputs["b2r"], np.float32)

    xf = x.reshape(-1, D)
    top_i, gw = _routing(xf, Wg, gate_bias)

    # balanced shard assignment -> baked 128-slot tiles per expert
    bins, eload = _balance(top_i)
    core_load = eload.reshape(N_CORES, 2, E).sum(axis=1)
    te = tuple(int(-(-int(core_load[:, e].max()) // 128)) for e in range(E))

    if _CACHE.get("te") != te:
        _CACHE["nc"] = _build_nc(te)
        _CACHE["te"] = te
    nc = _CACHE["nc"]

    # weight tensors (shared across cores, pre-scaled into fp16 range)
    w1_np = np.ascontiguousarray(
        (W1r * WSCALE).astype(np.float16).reshape(E, 8, 128, H)
        .transpose(0, 2, 1, 3))
    w2_np = np.ascontiguousarray(
        (W2r * WSCALE).astype(np.float16).reshape(E, 4, 128, 1024)
        .transpose(0, 2, 1, 3))
    w1s_np = np.ascontiguousarray(
        (W1s * WSCALE).astype(np.float16).reshape(8, 128, H).transpose(1, 0, 2))
    w2s_np = np.ascontiguousarray(
        (W2s * WSCALE).astype(np.float16).reshape(4, 128, 1024)
        .transpose(1, 0, 2))

    core_ids_tokens = []
    in_maps = []
    for core in range(N_CORES):
        ids = np.concatenate([bins[2 * core], bins[2 * core + 1]])
        core_ids_tokens.append(ids)
        m = _prep_core(xf[ids], top_i[ids], gw[ids], te)
        m.update({"w1": w1_np, "w2": w2_np, "w1s": w1s_np, "w2s": w2s_np})
        in_maps.append(m)

    trace = bool(int(os.environ.get("KERNEL_TRACE", "0")))
    res = run_bass_kernel_spmd(nc, in_maps, core_ids=list(range(N_CORES)),
                               trace=trace)
    _CACHE["last_results"] = res

    yf = np.empty((N, 1024), np.float32)
    for core in range(N_CORES):
        yf[core_ids_tokens[core]] = res.results[core]["y"][:T]

    # bias terms (zero in this problem's inputs; handled exactly if not)
    if b2s.any() or b2r.any():
        gdense = np.zeros((N, E), np.float32)
        np.put_along_axis(gdense, top_i, gw, axis=1)
        yf = yf + b2s[None, :] + gdense @ b2r

    return yf.reshape(B, S, D).astype(np.float32)
